# revision 7
# baseline (speedup 1.0000x reference)
"""AlphaFold-style OuterProductMean pair feature on 8 trn2 NeuronCores.

Computation (full shapes):
    x_left, x_right: (1, N=128, R=256, E=32) fp32
    outer[b,i,j,l,r] = sum_n x_left[b,n,i,l] * x_right[b,n,j,r]
    out = outer.reshape(1, R, R, E*E) @ W + b          # W: (1024, 128)

Sharding: row-shard the pair grid — core k owns i in [32k, 32k+32).
Each core receives its x_left row block, the full x_right, W, b
(all staged host-side; no collectives), and writes its (32, 256, 128)
output row block. Host concatenates.

Per-core kernel:
  stage 1 (bf16 matmuls, 1 cycle/row): for each i, r:
           outer_i[l, j] = xl[:, i, :].T @ xr[:, :, r]   (K=n=128)
           Four r's run concurrently via 4x column tiling (M=32 each)
           producing PSUM chunks (128 part = (r%4, l), 256 j) whose
           partition index matches rows r*32+l of the host-permuted W.
  stage 2: pair[d, (u j)] = sum_c Wp_chunk[c].T @ outer_chunk[c]
           (8 accumulating matmuls, K=128, N=512 = two i's of 256 j).
Output per core is (i, d, j); host transposes to (i, j, d).
"""

import os
import sys

if "/opt/trn_rl_repo" not in sys.path:
    sys.path.insert(0, "/opt/trn_rl_repo")

# The NTFF profile hook modules (antenv.axon_hooks / axon.trn) don't exist in
# this container; run_bass_kernel_spmd(trace=...) would crash trying them.
os.environ["BASS_NEVER_TRACE"] = "1"

from contextlib import ExitStack

import numpy as np

import concourse.bass as bass
import concourse.tile as tile
from concourse import bacc, mybir
from concourse.bass_utils import run_bass_kernel_spmd

N_CORES = 8
N = 128  # MSA depth (contraction dim)
R = 256  # residues
E = 32   # 1D embedding
D = 128  # 2D embedding
IB = R // N_CORES  # 32 rows of i per core
BENCH_REPS = 5
STAGE2_BF16 = True  # False -> float32r stage 2 (more precise, maybe slower)

_cached = None
last_results = None  # BassKernelResults of the most recent run (for test harness)


def _build(reps=1, stage2_bf16=STAGE2_BF16):
    f32 = mybir.dt.float32
    f32r = mybir.dt.float32r
    bf16 = mybir.dt.bfloat16
    s2dt = bf16 if stage2_bf16 else f32r

    nc = bacc.Bacc(None, target_bir_lowering=False, debug=False)

    xl_d = nc.dram_tensor("xl", [N, IB * E], bf16, kind="ExternalInput")    # [n, i*32+l]
    xr_d = nc.dram_tensor("xr", [N, 4, 8, R], bf16, kind="ExternalInput")   # [n, g, c, j]
    wp_d = nc.dram_tensor("wp", [D, 8 * D], s2dt, kind="ExternalInput")     # [p, c*128+d]
    out_d = nc.dram_tensor("out", [IB, D, R], f32, kind="ExternalOutput")   # [i, d, j]

    with tile.TileContext(nc) as tc, ExitStack() as ctx:
        const = ctx.enter_context(tc.tile_pool(name="const", bufs=1))
        xl_sb = const.tile([N, IB * E], bf16)
        xr_sb = const.tile([N, 4, 8, R], bf16)
        wp_sb = const.tile([D, 8 * D], s2dt)

        nc.sync.dma_start(xl_sb[:], xl_d[:])
        for q in range(8):
            nc.sync.dma_start(xr_sb[:, q // 2, 4 * (q % 2):4 * (q % 2) + 4, :],
                              xr_d[:, q // 2, 4 * (q % 2):4 * (q % 2) + 4, :])
        nc.sync.dma_start(wp_sb[:], wp_d[:])

        outer_pool = ctx.enter_context(tc.tile_pool(name="outer", bufs=3))
        ps1 = ctx.enter_context(tc.tile_pool(name="ps1", bufs=3, space="PSUM"))
        ps2 = ctx.enter_context(tc.tile_pool(name="ps2", bufs=2, space="PSUM"))
        osb_pool = ctx.enter_context(tc.tile_pool(name="osb", bufs=4))

        evac_idx = 0
        for _rep in range(reps):
            for ip in range(IB // 2):  # pairs of i rows
                outer = outer_pool.tile([D, 8, 2, R], s2dt)  # (p, c, u, j)
                for u in range(2):
                    i = 2 * ip + u
                    for cpp in range(2):  # 4 chunks share a 2-bank PSUM tile
                        p1 = ps1.tile([D, 2, 2, R], f32)
                        for cp2 in range(2):
                            for g in range(4):
                                nc.tensor.matmul(
                                    p1[32 * g:32 * g + 32, cp2, :, :],
                                    xl_sb[:, E * i:E * i + E],
                                    xr_sb[:, g, 4 * cpp + 2 * cp2:
                                          4 * cpp + 2 * cp2 + 2, :],
                                    start=True,
                                    stop=True,
                                    tile_position=(0, 32 * g),
                                )
                        # FD=1024 PSUM evacuation, alternating ACT/DVE 1:1
                        dst = outer[:, 4 * cpp:4 * cpp + 4, u, :]
                        if evac_idx % 2 == 0:
                            nc.scalar.copy(dst, p1[:])
                        else:
                            nc.vector.tensor_copy(dst, p1[:])
                        evac_idx += 1

                p2 = ps2.tile([D, 2, R], f32)
                for c in range(8):
                    nc.tensor.matmul(
                        p2[:],
                        wp_sb[:, D * c:D * c + D],
                        outer[:, c],
                        start=(c == 0),
                        stop=(c == 7),
                    )
                # bias is added host-side (free); plain copy to SBUF staging,
                # sharing the DVE/ACT 2:1 rotation with the stage-1 evacs
                osb = osb_pool.tile([D, 2, R], f32)
                if evac_idx % 3 < 2:
                    nc.vector.tensor_copy(osb[:], p2[:])
                else:
                    nc.scalar.copy(osb[:], p2[:])
                evac_idx += 1
                nc.sync.dma_start(out_d[2 * ip], osb[:, 0, :])
                nc.sync.dma_start(out_d[2 * ip + 1], osb[:, 1, :])

    nc.compile()
    return nc


def make_in_maps(x_left, x_right, W, b, stage2_bf16=STAGE2_BF16):
    import ml_dtypes

    xl = np.asarray(x_left, dtype=np.float32)[0]   # (n, i, l)
    xr = np.asarray(x_right, dtype=np.float32)[0]  # (n, j, r)
    W = np.asarray(W, dtype=np.float32)
    b = np.asarray(b, dtype=np.float32)

    xl = np.ascontiguousarray(xl).astype(ml_dtypes.bfloat16)
    xr_rj = xr.transpose(0, 2, 1)                      # [n, r, j]
    xr_flat = np.ascontiguousarray(
        xr_rj.reshape(N, 8, 4, R).transpose(0, 2, 1, 3).astype(ml_dtypes.bfloat16)
    )  # [n, g, c, j], r = 4c+g
    # W[(l*32+r), d] -> W_perm[(r*32+l), d] -> chunk-major sbuf layout [p, c*128+d]
    wp = (
        W.reshape(E, E, D).transpose(1, 0, 2).reshape(8, D, D)
        .transpose(1, 0, 2).reshape(D, 8 * D)
    )
    wp = np.ascontiguousarray(wp)
    if stage2_bf16:
        wp = wp.astype(ml_dtypes.bfloat16)

    in_maps = []
    for k in range(N_CORES):
        xlk = np.ascontiguousarray(xl[:, IB * k:IB * (k + 1), :]).reshape(N, IB * E)
        in_maps.append({"xl": xlk, "xr": xr_flat, "wp": wp})
    return in_maps


def kernel(x_left, x_right, W, b):
    global _cached, last_results
    if _cached is None:
        _cached = _build()
    nc = _cached

    in_maps = make_in_maps(x_left, x_right, W, b)
    res = run_bass_kernel_spmd(nc, in_maps, list(range(N_CORES)))
    last_results = res

    blocks = [res.results[k]["out"].transpose(0, 2, 1) for k in range(N_CORES)]
    out = np.concatenate(blocks, axis=0)[None]  # (1, 256, 256, 128)
    out += np.asarray(b, dtype=np.float32)  # bias broadcast over d (host-side)
    return out



# revision 8
# speedup vs baseline: 1.7750x; 1.7750x over previous
"""AlphaFold-style OuterProductMean pair feature on 8 trn2 NeuronCores.

Computation (full shapes):
    x_left, x_right: (1, N=128, R=256, E=32) fp32
    outer[b,i,j,l,r] = sum_n x_left[b,n,i,l] * x_right[b,n,j,r]
    out = outer.reshape(1, R, R, E*E) @ W + b          # W: (1024, 128)

Sharding: row-shard the pair grid — core k owns i in [32k, 32k+32).
Each core receives its x_left row block, the full x_right, W, b
(all staged host-side; no collectives), and writes its (32, 256, 128)
output row block. Host concatenates.

Per-core kernel:
  stage 1 (bf16 matmuls, 1 cycle/row): for each i, r:
           outer_i[l, j] = xl[:, i, :].T @ xr[:, :, r]   (K=n=128)
           Four r's run concurrently via 4x column tiling (M=32 each)
           producing PSUM chunks (128 part = (r%4, l), 256 j) whose
           partition index matches rows r*32+l of the host-permuted W.
  stage 2: pair[d, (u j)] = sum_c Wp_chunk[c].T @ outer_chunk[c]
           (8 accumulating matmuls, K=128, N=512 = two i's of 256 j).
Output per core is (i, d, j); host transposes to (i, j, d).
"""

import os
import sys

if "/opt/trn_rl_repo" not in sys.path:
    sys.path.insert(0, "/opt/trn_rl_repo")

# The NTFF profile hook modules (antenv.axon_hooks / axon.trn) don't exist in
# this container; run_bass_kernel_spmd(trace=...) would crash trying them.
os.environ["BASS_NEVER_TRACE"] = "1"

from contextlib import ExitStack

import numpy as np

import concourse.bass as bass
import concourse.tile as tile
from concourse import bacc, mybir
from concourse.bass_utils import run_bass_kernel_spmd

N_CORES = 8
N = 128  # MSA depth (contraction dim)
R = 256  # residues
E = 32   # 1D embedding
D = 128  # 2D embedding
IB = R // N_CORES  # 32 rows of i per core
BENCH_REPS = 5
STAGE2_BF16 = True  # False -> float32r stage 2 (more precise, maybe slower)

_cached = None
last_results = None  # BassKernelResults of the most recent run (for test harness)


def _build(reps=1, stage2_bf16=STAGE2_BF16):
    f32 = mybir.dt.float32
    f32r = mybir.dt.float32r
    bf16 = mybir.dt.bfloat16
    s2dt = bf16 if stage2_bf16 else f32r

    nc = bacc.Bacc(None, target_bir_lowering=False, debug=False)

    xl_d = nc.dram_tensor("xl", [N, IB * E], bf16, kind="ExternalInput")    # [n, i*32+l]
    xr_d = nc.dram_tensor("xr", [N, 4, 8, R], bf16, kind="ExternalInput")   # [n, g, c, j]
    wp_d = nc.dram_tensor("wp", [D, 8 * D], s2dt, kind="ExternalInput")     # [p, c*128+d]
    out_d = nc.dram_tensor("out", [IB, D, R], bf16, kind="ExternalOutput")  # [i, d, j]

    with tile.TileContext(nc) as tc, ExitStack() as ctx:
        const = ctx.enter_context(tc.tile_pool(name="const", bufs=1))
        xl_sb = const.tile([N, IB * E], bf16)
        xr_sb = const.tile([N, 4, 8, R], bf16)
        wp_sb = const.tile([D, 8 * D], s2dt)

        nc.sync.dma_start(xl_sb[:], xl_d[:])
        for q in range(8):
            nc.sync.dma_start(xr_sb[:, q // 2, 4 * (q % 2):4 * (q % 2) + 4, :],
                              xr_d[:, q // 2, 4 * (q % 2):4 * (q % 2) + 4, :])
        nc.sync.dma_start(wp_sb[:], wp_d[:])

        outer_pool = ctx.enter_context(tc.tile_pool(name="outer", bufs=3))
        ps1 = ctx.enter_context(tc.tile_pool(name="ps1", bufs=3, space="PSUM"))
        ps2 = ctx.enter_context(tc.tile_pool(name="ps2", bufs=2, space="PSUM"))
        osb_pool = ctx.enter_context(tc.tile_pool(name="osb", bufs=4))

        evac_idx = 0
        for _rep in range(reps):
            for ip in range(IB // 2):  # pairs of i rows
                outer = outer_pool.tile([D, 8, 2, R], s2dt)  # (p, c, u, j)
                for u in range(2):
                    i = 2 * ip + u
                    for cpp in range(2):  # 4 chunks share a 2-bank PSUM tile
                        p1 = ps1.tile([D, 2, 2, R], f32)
                        for cp2 in range(2):
                            for g in range(4):
                                nc.tensor.matmul(
                                    p1[32 * g:32 * g + 32, cp2, :, :],
                                    xl_sb[:, E * i:E * i + E],
                                    xr_sb[:, g, 4 * cpp + 2 * cp2:
                                          4 * cpp + 2 * cp2 + 2, :],
                                    start=True,
                                    stop=True,
                                    tile_position=(0, 32 * g),
                                )
                        # FD=1024 PSUM evacuation, alternating ACT/DVE 1:1
                        dst = outer[:, 4 * cpp:4 * cpp + 4, u, :]
                        if evac_idx % 2 == 0:
                            nc.scalar.copy(dst, p1[:])
                        else:
                            nc.vector.tensor_copy(dst, p1[:])
                        evac_idx += 1

                p2 = ps2.tile([D, 2, R], f32)
                for c in range(8):
                    nc.tensor.matmul(
                        p2[:],
                        wp_sb[:, D * c:D * c + D],
                        outer[:, c],
                        start=(c == 0),
                        stop=(c == 7),
                    )
                # bias is added host-side (free); plain copy to SBUF staging,
                # sharing the DVE/ACT 2:1 rotation with the stage-1 evacs
                osb = osb_pool.tile([D, 2, R], bf16)
                if evac_idx % 3 < 2:
                    nc.vector.tensor_copy(osb[:], p2[:])
                else:
                    nc.scalar.copy(osb[:], p2[:])
                evac_idx += 1
                nc.sync.dma_start(out_d[2 * ip], osb[:, 0, :])
                nc.sync.dma_start(out_d[2 * ip + 1], osb[:, 1, :])

    nc.compile()
    return nc


def make_in_maps(x_left, x_right, W, b, stage2_bf16=STAGE2_BF16):
    import ml_dtypes

    xl = np.asarray(x_left, dtype=np.float32)[0]   # (n, i, l)
    xr = np.asarray(x_right, dtype=np.float32)[0]  # (n, j, r)
    W = np.asarray(W, dtype=np.float32)
    b = np.asarray(b, dtype=np.float32)

    xl = np.ascontiguousarray(xl).astype(ml_dtypes.bfloat16)
    xr_rj = xr.transpose(0, 2, 1)                      # [n, r, j]
    xr_flat = np.ascontiguousarray(
        xr_rj.reshape(N, 8, 4, R).transpose(0, 2, 1, 3).astype(ml_dtypes.bfloat16)
    )  # [n, g, c, j], r = 4c+g
    # W[(l*32+r), d] -> W_perm[(r*32+l), d] -> chunk-major sbuf layout [p, c*128+d]
    wp = (
        W.reshape(E, E, D).transpose(1, 0, 2).reshape(8, D, D)
        .transpose(1, 0, 2).reshape(D, 8 * D)
    )
    wp = np.ascontiguousarray(wp)
    if stage2_bf16:
        wp = wp.astype(ml_dtypes.bfloat16)

    in_maps = []
    for k in range(N_CORES):
        xlk = np.ascontiguousarray(xl[:, IB * k:IB * (k + 1), :]).reshape(N, IB * E)
        in_maps.append({"xl": xlk, "xr": xr_flat, "wp": wp})
    return in_maps


def kernel(x_left, x_right, W, b):
    global _cached, last_results
    if _cached is None:
        _cached = _build()
    nc = _cached

    in_maps = make_in_maps(x_left, x_right, W, b)
    res = run_bass_kernel_spmd(nc, in_maps, list(range(N_CORES)))
    last_results = res

    blocks = [
        np.asarray(res.results[k]["out"], dtype=np.float32).transpose(0, 2, 1)
        for k in range(N_CORES)
    ]
    out = np.concatenate(blocks, axis=0)[None]  # (1, 256, 256, 128)
    out += np.asarray(b, dtype=np.float32)  # bias broadcast over d (host-side)
    return out



# revision 9
# speedup vs baseline: 3.1328x; 1.7650x over previous
"""AlphaFold-style OuterProductMean pair feature on 8 trn2 NeuronCores.

Computation (full shapes):
    x_left, x_right: (1, N=128, R=256, E=32) fp32
    outer[b,i,j,l,r] = sum_n x_left[b,n,i,l] * x_right[b,n,j,r]
    out = outer.reshape(1, R, R, E*E) @ W + b          # W: (1024, 128)

Sharding: row-shard the pair grid — core k owns i in [32k, 32k+32).
Each core receives its x_left row block, the full x_right, W, b
(all staged host-side; no collectives), and writes its (32, 256, 128)
output row block. Host concatenates.

Per-core kernel:
  stage 1 (bf16 matmuls): for each i: 16 matmuls of K=n=128, M=32,
           f=512. The four col-tiles g=r%4 run concurrently
           (tile_position=(0,32g)); each streams xr[n, g, c-pair, j]
           (host layout [n, g, c, j], r = 4c+g) so one matmul covers
           two r-chunks. PSUM partition = (r%4)*32+l, matching rows
           r*32+l of the host-permuted W. 2-bank PSUM tiles are
           evacuated as FD=1024 copies alternating ACT/DVE 1:1.
  stage 2: pair[d, (u j)] = sum_c Wp_chunk[c].T @ outer_chunk[c]
           (8 accumulating matmuls, K=128, N=512 = two i's of 256 j).
Output per core is (i, d, j) bf16; host converts and transposes.
"""

import os
import sys

if "/opt/trn_rl_repo" not in sys.path:
    sys.path.insert(0, "/opt/trn_rl_repo")

# The NTFF profile hook modules (antenv.axon_hooks / axon.trn) don't exist in
# this container; run_bass_kernel_spmd(trace=...) would crash trying them.
os.environ["BASS_NEVER_TRACE"] = "1"

from contextlib import ExitStack

import numpy as np

import concourse.bass as bass
import concourse.tile as tile
from concourse import bacc, mybir
from concourse.bass_utils import run_bass_kernel_spmd

N_CORES = 8
N = 128  # MSA depth (contraction dim)
R = 256  # residues
E = 32   # 1D embedding
D = 128  # 2D embedding
IB = R // N_CORES  # 32 rows of i per core
BENCH_REPS = 5
STAGE2_BF16 = True  # False -> float32r stage 2 (more precise, maybe slower)

_cached = None
last_results = None  # BassKernelResults of the most recent run (for test harness)


def _build(reps=1, stage2_bf16=STAGE2_BF16):
    f32 = mybir.dt.float32
    f32r = mybir.dt.float32r
    bf16 = mybir.dt.bfloat16
    s2dt = bf16 if stage2_bf16 else f32r

    nc = bacc.Bacc(None, target_bir_lowering=False, debug=False)

    xl_d = nc.dram_tensor("xl", [N, IB * E], bf16, kind="ExternalInput")    # [n, i*32+l]
    xr_d = nc.dram_tensor("xr", [N, 4, 8, R], bf16, kind="ExternalInput")   # [n, g, c, j]
    wp_d = nc.dram_tensor("wp", [D, 8 * D], s2dt, kind="ExternalInput")     # [p, c*128+d]
    out_d = nc.dram_tensor("out", [IB, D, R], bf16, kind="ExternalOutput")  # [i, d, j]

    with tile.TileContext(nc) as tc, ExitStack() as ctx:
        const = ctx.enter_context(tc.tile_pool(name="const", bufs=1))
        xl_sb = const.tile([N, IB * E], bf16)
        xr_sb = const.tile([N, 4, 8, R], bf16)
        wp_sb = const.tile([D, 8 * D], s2dt)

        nc.sync.dma_start(xl_sb[:], xl_d[:])
        for q in range(8):
            nc.sync.dma_start(xr_sb[:, q // 2, 4 * (q % 2):4 * (q % 2) + 4, :],
                              xr_d[:, q // 2, 4 * (q % 2):4 * (q % 2) + 4, :])
        nc.sync.dma_start(wp_sb[:], wp_d[:])

        outer_pool = ctx.enter_context(tc.tile_pool(name="outer", bufs=3))
        ps1 = ctx.enter_context(tc.tile_pool(name="ps1", bufs=3, space="PSUM"))
        ps2 = ctx.enter_context(tc.tile_pool(name="ps2", bufs=2, space="PSUM"))
        osb_pool = ctx.enter_context(tc.tile_pool(name="osb", bufs=4))

        evac_idx = 0
        for _rep in range(reps):
            for ip in range(IB // 2):  # pairs of i rows
                outer = outer_pool.tile([D, 8, 2, R], s2dt)  # (p, c, u, j)
                for u in range(2):
                    i = 2 * ip + u
                    for cpp in range(2):  # 4 chunks share a 2-bank PSUM tile
                        p1 = ps1.tile([D, 2, 2, R], f32)
                        for cp2 in range(2):
                            for g in range(4):
                                nc.tensor.matmul(
                                    p1[32 * g:32 * g + 32, cp2, :, :],
                                    xl_sb[:, E * i:E * i + E],
                                    xr_sb[:, g, 4 * cpp + 2 * cp2:
                                          4 * cpp + 2 * cp2 + 2, :],
                                    start=True,
                                    stop=True,
                                    tile_position=(0, 32 * g),
                                )
                        # FD=1024 PSUM evacuation, alternating ACT/DVE 1:1
                        dst = outer[:, 4 * cpp:4 * cpp + 4, u, :]
                        if evac_idx % 2 == 0:
                            nc.scalar.copy(dst, p1[:])
                        else:
                            nc.vector.tensor_copy(dst, p1[:])
                        evac_idx += 1

                p2 = ps2.tile([D, 2, R], f32)
                for c in range(8):
                    nc.tensor.matmul(
                        p2[:],
                        wp_sb[:, D * c:D * c + D],
                        outer[:, c],
                        start=(c == 0),
                        stop=(c == 7),
                    )
                # bias is added host-side (free); plain copy to SBUF staging,
                # sharing the DVE/ACT 2:1 rotation with the stage-1 evacs
                osb = osb_pool.tile([D, 2, R], bf16)
                if evac_idx % 3 < 2:
                    nc.vector.tensor_copy(osb[:], p2[:])
                else:
                    nc.scalar.copy(osb[:], p2[:])
                evac_idx += 1
                nc.sync.dma_start(out_d[2 * ip], osb[:, 0, :])
                nc.sync.dma_start(out_d[2 * ip + 1], osb[:, 1, :])

    nc.compile()
    return nc


def make_in_maps(x_left, x_right, W, b, stage2_bf16=STAGE2_BF16):
    import ml_dtypes

    xl = np.asarray(x_left, dtype=np.float32)[0]   # (n, i, l)
    xr = np.asarray(x_right, dtype=np.float32)[0]  # (n, j, r)
    W = np.asarray(W, dtype=np.float32)
    b = np.asarray(b, dtype=np.float32)

    xl = np.ascontiguousarray(xl).astype(ml_dtypes.bfloat16)
    xr_rj = xr.transpose(0, 2, 1)                      # [n, r, j]
    xr_flat = np.ascontiguousarray(
        xr_rj.reshape(N, 8, 4, R).transpose(0, 2, 1, 3).astype(ml_dtypes.bfloat16)
    )  # [n, g, c, j], r = 4c+g
    # W[(l*32+r), d] -> W_perm[(r*32+l), d] -> chunk-major sbuf layout [p, c*128+d]
    wp = (
        W.reshape(E, E, D).transpose(1, 0, 2).reshape(8, D, D)
        .transpose(1, 0, 2).reshape(D, 8 * D)
    )
    wp = np.ascontiguousarray(wp)
    if stage2_bf16:
        wp = wp.astype(ml_dtypes.bfloat16)

    in_maps = []
    for k in range(N_CORES):
        xlk = np.ascontiguousarray(xl[:, IB * k:IB * (k + 1), :]).reshape(N, IB * E)
        in_maps.append({"xl": xlk, "xr": xr_flat, "wp": wp})
    return in_maps


def kernel(x_left, x_right, W, b):
    global _cached, last_results
    if _cached is None:
        _cached = _build()
    nc = _cached

    in_maps = make_in_maps(x_left, x_right, W, b)
    res = run_bass_kernel_spmd(nc, in_maps, list(range(N_CORES)))
    last_results = res

    blocks = [
        np.asarray(res.results[k]["out"], dtype=np.float32).transpose(0, 2, 1)
        for k in range(N_CORES)
    ]
    out = np.concatenate(blocks, axis=0)[None]  # (1, 256, 256, 128)
    out += np.asarray(b, dtype=np.float32)  # bias broadcast over d (host-side)
    return out

